# revision 1
# baseline (speedup 1.0000x reference)
"""Multi-head attention (B=2, S=4096, DM=512, H=8) on 8 trn2 NeuronCores.

Sharding: data + head parallel. Core c handles batch b = c//4 and head pair
hp = c%4 (heads 2hp, 2hp+1 = a 128-wide slice of the model dim). Each core
computes its two heads' full attention plus the partial output projection
(its 128 rows of Wo); the host sums the 4 partials per batch and adds bo.

Device pipeline (per core):
  qT,kT,vT  [128,ch,t,512] bf16 host-pretransposed, chunk-major (4KB DMA rows)
  QhT,KhT   [128, 4096] bf16   projected, transposed: rows 0:64 head0, 64:128 head1
  VA        [128, 32, 130]     V blocks + a ones column per head (rowsum of the
                               attention falls out of the AV matmul's 65th row)
  logits^T  [128sk, 1024] PSUM both heads side by side (K=64 QK pairs); softmax
                               exp runs unbiased: the padding mask is applied by
                               zeroing masked VA rows (incl. the ones column), so
                               masked keys contribute 0 to AV and to the rowsum.
                               3 of 4 exps run on ScalarE; every 4th sk block uses
                               a one-op Schraudolph fast-exp on VectorE (int16
                               bitcast to bf16) to balance the two engines.
  acc_h     [65, 512] PSUM     AV accumulation over 32 sk blocks; row 64 = rowsum
  Wo        per head, K=64 matmuls; 1/rowsum applied per-partition (q) on the
                               [q,512] outputs, combined via scalar_tensor_tensor

K/V projections for s-group g are emitted interleaved with attention chunk 0's
j-group g so the PE stream reaches the first QK as soon as the first DMAs land.
"""
import numpy as np
import ml_dtypes

import concourse.bass as bass
from concourse import bacc
import concourse.mybir as mybir
import concourse.tile as tile
from concourse import bass_utils
from concourse.alu_op_type import AluOpType

FP32 = mybir.dt.float32
BF16 = mybir.dt.bfloat16
AF = mybir.ActivationFunctionType

B, S, DM, H = 2, 4096, 512, 8
D = DM // H              # 64
NCORES = 8
CHUNK = 512              # q columns processed per attention chunk
NCH = S // CHUNK         # 8
NSK = S // 128           # 32 sk blocks
NT = DM // 128           # 4 dm tiles

_CACHE = {}


def _build(with_bias):
    nc = bacc.Bacc("TRN2", target_bir_lowering=False, debug=False)

    # chunk-major: [ch, p, t, c] so each chunk's DMA has 4KB-contiguous rows
    qT = nc.dram_tensor("qT", [NCH, 128, NT, CHUNK], BF16, kind="ExternalInput")
    kT = nc.dram_tensor("kT", [NCH, 128, NT, CHUNK], BF16, kind="ExternalInput")
    vT = nc.dram_tensor("vT", [NCH, 128, NT, CHUNK], BF16, kind="ExternalInput")
    m01 = nc.dram_tensor("m01", [128, NSK], FP32, kind="ExternalInput")
    wq = nc.dram_tensor("wq", [DM, 128], BF16, kind="ExternalInput")
    wk = nc.dram_tensor("wk", [DM, 128], BF16, kind="ExternalInput")
    wv = nc.dram_tensor("wv", [DM, 130], BF16, kind="ExternalInput")
    bqk = nc.dram_tensor("bqk", [1, 256], BF16, kind="ExternalInput")  # bq|bk
    bv = nc.dram_tensor("bv", [1, 130], BF16, kind="ExternalInput")
    wo = nc.dram_tensor("wo", [128, DM], BF16, kind="ExternalInput")
    out = nc.dram_tensor("out", [S, DM], FP32, kind="ExternalOutput")

    with tile.TileContext(nc) as tc:
        with tc.tile_pool(name="consts", bufs=1) as consts, \
             tc.tile_pool(name="acts", bufs=1) as acts:
            # ---- first chunk's activations land before anything else ----
            qT_sb = acts.tile([128, NCH, NT, CHUNK], BF16)
            kT_sb = acts.tile([128, NCH, NT, CHUNK], BF16)
            vT_sb = acts.tile([128, NCH, NT, CHUNK], BF16)
            nc.sync.dma_start(out=kT_sb[:, 0], in_=kT[0])
            nc.sync.dma_start(out=vT_sb[:, 0], in_=vT[0])
            nc.sync.dma_start(out=qT_sb[:, 0], in_=qT[0])

            # ---- tiny constants; warm the Exp table set during the DMA phase ----
            ones_sb = consts.tile([1, CHUNK], BF16)
            nc.vector.memset(ones_sb, 1.0)
            warm = consts.tile([1, 1], FP32)
            nc.scalar.activation(warm, ones_sb[0:1, 0:1], AF.Exp)

            wq_sb = consts.tile([128, NT, 128], BF16)
            wk_sb = consts.tile([128, NT, 128], BF16)
            wv_sb = consts.tile([128, NT, 130], BF16)
            bqk_sb = consts.tile([1, 256], BF16)
            bv_sb = consts.tile([1, 130], BF16)
            wo_sb = consts.tile([64, 2, DM], BF16)   # [64, h, 512] both heads base-0
            m01_sb = consts.tile([128, NSK], FP32)
            for t in range(NT):
                nc.sync.dma_start(out=wk_sb[:, t, :], in_=wk[t * 128:(t + 1) * 128, :])
                nc.sync.dma_start(out=wv_sb[:, t, :], in_=wv[t * 128:(t + 1) * 128, :])
                nc.sync.dma_start(out=wq_sb[:, t, :], in_=wq[t * 128:(t + 1) * 128, :])
            nc.sync.dma_start(out=bv_sb, in_=bv[:, :])
            nc.sync.dma_start(out=m01_sb, in_=m01[:, :])
            if with_bias:
                nc.sync.dma_start(out=bqk_sb, in_=bqk[:, :])

            # ---- remaining activation DMAs ----

            for h in range(2):
                nc.sync.dma_start(out=wo_sb[:, h, :], in_=wo[h * 64:(h + 1) * 64, :])
            for ch in range(1, NCH):
                nc.sync.dma_start(out=kT_sb[:, ch], in_=kT[ch])
                nc.sync.dma_start(out=vT_sb[:, ch], in_=vT[ch])
            for ch in range(1, NCH):
                nc.sync.dma_start(out=qT_sb[:, ch], in_=qT[ch])

            QhT = acts.tile([128, S], BF16)
            KhT = acts.tile([128, S], BF16)
            VA = acts.tile([128, NSK, 130], BF16)
            outT0 = acts.tile([64, S], BF16)
            outT1 = acts.tile([64, S], BF16)

            with tc.tile_pool(name="pproj", bufs=2, space="PSUM") as pp, \
                 tc.tile_pool(name="plog", bufs=2, space="PSUM") as plog, \
                 tc.tile_pool(name="pacc", bufs=1, space="PSUM") as pacc, \
                 tc.tile_pool(name="sexp", bufs=4) as sexp, \
                 tc.tile_pool(name="sout", bufs=3) as sout, \
                 tc.tile_pool(name="srs", bufs=2) as srs:

                def proj_qk(dst, w_sb, brow, x_sb, ch):
                    ps = pp.tile([128, CHUNK], FP32, tag="psqk")
                    sl = bass.ds(ch * CHUNK, CHUNK)
                    for t in range(NT):
                        nc.tensor.matmul(ps, w_sb[:, t, :], x_sb[:, ch, t, :],
                                         start=(t == 0),
                                         stop=(t == NT - 1 and not with_bias))
                    if with_bias:
                        nc.tensor.matmul(ps, brow, ones_sb, start=False, stop=True)
                    nc.vector.tensor_copy(dst[:, sl], ps)

                def proj_k(ch):
                    proj_qk(KhT, wk_sb, bqk_sb[0:1, 128:256], kT_sb, ch)

                def proj_v(j):
                    # V bias matmul always runs: it also writes the ones columns
                    # (cols 64/129) that produce the attention rowsums.
                    psv = pp.tile([128, CHUNK], FP32, tag="psqk")
                    for t in range(NT):
                        nc.tensor.matmul(psv[:, 0:130],
                                         vT_sb[:, j // 4, t,
                                               (j % 4) * 128:(j % 4 + 1) * 128],
                                         wv_sb[:, t, :],
                                         start=(t == 0), stop=False)
                    nc.tensor.matmul(psv[:, 0:130], ones_sb[0:1, 0:128], bv_sb,
                                     start=False, stop=True)
                    nc.vector.tensor_scalar(VA[:, j, :], psv[:, 0:130],
                                            m01_sb[:, j:j + 1], None,
                                            op0=AluOpType.mult)

                # Schraudolph fast-exp in bf16 domain (DVE path):
                # exp(x) ~= bitcast_bf16(int16(x * 2^7/ln2 + (127*2^7 - C)))
                EXP_A = 184.6650292
                EXP_B = float(127 * (1 << 7)) - 5.5918

                def attn_j(j, qsl, acc0, acc1, on_dve):
                    ksl = bass.ds(j * 128, 128)
                    pt = plog.tile([128, 2 * CHUNK], FP32, tag="logits")
                    nc.tensor.matmul(pt[:, 0:CHUNK],
                                     KhT[0:64, ksl], QhT[0:64, qsl],
                                     start=True, stop=True)
                    nc.tensor.matmul(pt[:, CHUNK:2 * CHUNK],
                                     KhT[64:128, ksl], QhT[64:128, qsl],
                                     start=True, stop=True)
                    if on_dve:
                        ei = sexp.tile([128, 2 * CHUNK], mybir.dt.int16, tag="expT")
                        nc.vector.tensor_scalar(ei, pt, EXP_A, EXP_B,
                                                op0=AluOpType.mult,
                                                op1=AluOpType.add)
                        et = ei.bitcast(BF16)
                    else:
                        et = sexp.tile([128, 2 * CHUNK], BF16, tag="expT")
                        nc.scalar.activation(et, pt, AF.Exp)
                    nc.tensor.matmul(acc0, VA[:, j, 0:65], et[:, 0:CHUNK],
                                     start=(j == 0), stop=(j == NSK - 1))
                    nc.tensor.matmul(acc1, VA[:, j, 65:130], et[:, CHUNK:2 * CHUNK],
                                     start=(j == 0), stop=(j == NSK - 1))

                def wo_combine(rti, ch, qt):
                    gq = ch * 4 + qt
                    lsl = bass.ds(gq * 128, 128)
                    pso0 = pp.tile([128, DM], FP32, tag="psqk")
                    pso1 = pp.tile([128, DM], FP32, tag="psqk")
                    nc.tensor.matmul(pso0, outT0[:, lsl], wo_sb[:, 0, :],
                                     start=True, stop=True)
                    nc.tensor.matmul(pso1, outT1[:, lsl], wo_sb[:, 1, :],
                                     start=True, stop=True)
                    tmp = sout.tile([128, DM], FP32, tag="tmp")
                    nc.vector.tensor_scalar(tmp, pso0, rti[:, qt:qt + 1], None,
                                            op0=AluOpType.mult)
                    ot = sout.tile([128, DM], FP32, tag="ot")
                    nc.vector.scalar_tensor_tensor(
                        ot, pso1, rti[:, 4 + qt:5 + qt], tmp,
                        op0=AluOpType.mult, op1=AluOpType.add)
                    nc.sync.dma_start(out=out[gq * 128:(gq + 1) * 128, :], in_=ot)

                pending = None
                for ch in range(NCH):
                    qsl = bass.ds(ch * CHUNK, CHUNK)
                    if ch == 0:
                        proj_k(0)
                        for j in range(4):
                            proj_v(j)
                        proj_qk(QhT, wq_sb, bqk_sb[0:1, 0:128], qT_sb, 0)
                    if ch + 1 < NCH:
                        proj_qk(QhT, wq_sb, bqk_sb[0:1, 0:128], qT_sb, ch + 1)

                    acc0 = pacc.tile([65, CHUNK], FP32, tag="acc0")
                    acc1 = pacc.tile([65, CHUNK], FP32, tag="acc1")
                    for j in range(NSK):
                        if ch == 0 and j % 4 == 0 and j > 0:
                            # feed the rest of the K/V projections just in time
                            g = j // 4
                            proj_k(g)
                            for jj in range(4 * g, 4 * g + 4):
                                proj_v(jj)
                        if pending is not None and j % 8 == 4:
                            wo_combine(pending[0], pending[1], (j - 4) // 8)
                        attn_j(j, qsl, acc0, acc1, on_dve=(j % 4 == 2))

                    # stage acc (one copy each -> early PSUM release), then
                    # derive bf16 outT and the rowsum row from the staging tile
                    stg = srs.tile([65, 2 * CHUNK], FP32, tag="stg")
                    nc.vector.tensor_copy(stg[:, 0:CHUNK], acc0)
                    nc.vector.tensor_copy(stg[:, CHUNK:2 * CHUNK], acc1)
                    nc.vector.tensor_copy(outT0[:, qsl], stg[0:64, 0:CHUNK])
                    nc.vector.tensor_copy(outT1[:, qsl], stg[0:64, CHUNK:2 * CHUNK])
                    # transpose rowsums to partitions: rt[p, h*4+qt] = rs_h[qt*128+p]
                    rt = srs.tile([128, 8], FP32, tag="rt")
                    for h in range(2):
                        for qt in range(4):
                            nc.sync.dma_start(
                                out=rt[:, h * 4 + qt:h * 4 + qt + 1],
                                in_=stg[64:65, h * CHUNK + qt * 128:
                                        h * CHUNK + (qt + 1) * 128])
                    rti = srs.tile([128, 8], FP32, tag="rti")
                    nc.vector.reciprocal(rti, rt)
                    pending = (rti, ch)
                for qt in range(4):
                    wo_combine(pending[0], pending[1], qt)
    nc.compile()
    return nc


def _prep_core_inputs(c, q, k, v, mask, Wq, bq, Wk, bk, Wv, bv, Wo):
    b, hp = divmod(c, 4)
    sl = slice(hp * 128, (hp + 1) * 128)
    bf = ml_dtypes.bfloat16
    scale = 1.0 / np.sqrt(np.float32(D))

    def packT(x):
        # [S, DM] -> transpose -> [NCH, 128, NT, CHUNK] chunk-major contiguous
        xt = x.T.reshape(NT, 128, NCH, CHUNK).transpose(2, 1, 0, 3)
        return np.ascontiguousarray(xt).astype(bf)
    qTb = packT(q[b])
    kTb = packT(k[b])
    vTb = packT(v[b])
    m01c = np.ascontiguousarray(
        (np.float32(1.0) - mask[b, 0, 0, :]).reshape(NSK, 128).T
    ).astype(np.float32)

    wq_c = np.ascontiguousarray(Wq[:, sl] * scale).astype(bf)
    wk_c = np.ascontiguousarray(Wk[:, sl]).astype(bf)
    wvs = Wv[:, sl]
    wv_c = np.zeros((DM, 130), np.float32)
    wv_c[:, 0:64] = wvs[:, 0:64]
    wv_c[:, 65:129] = wvs[:, 64:128]
    wv_c = wv_c.astype(bf)
    bqk_c = np.concatenate([bq[sl] * scale, bk[sl]]).reshape(1, 256).astype(bf)
    bv_c = np.zeros((1, 130), np.float32)
    bv_c[0, 0:64] = bv[sl][0:64]
    bv_c[0, 64] = 1.0
    bv_c[0, 65:129] = bv[sl][64:128]
    bv_c[0, 129] = 1.0
    bv_c = bv_c.astype(bf)
    wo_c = np.ascontiguousarray(Wo[sl, :]).astype(bf)
    return {"qT": qTb, "kT": kTb, "vT": vTb, "m01": m01c, "wq": wq_c, "wk": wk_c,
            "wv": wv_c, "bqk": bqk_c, "bv": bv_c, "wo": wo_c}


LAST_RESULT = None


def kernel(q, k, v, mask, Wq, bq, Wk, bk, Wv, bv, Wo, bo):
    global LAST_RESULT
    f32 = np.float32
    q, k, v, mask = (np.asarray(x, f32) for x in (q, k, v, mask))
    Wq, bq, Wk, bk, Wv, bv, Wo, bo = (
        np.asarray(x, f32) for x in (Wq, bq, Wk, bk, Wv, bv, Wo, bo))

    with_bias = bool(np.any(bq) or np.any(bk) or np.any(bv))
    key = ("nc", with_bias)
    if key not in _CACHE:
        _CACHE[key] = _build(with_bias)
    nc = _CACHE[key]

    in_maps = [_prep_core_inputs(c, q, k, v, mask, Wq, bq, Wk, bk, Wv, bv, Wo)
               for c in range(NCORES)]
    res = bass_utils.run_bass_kernel_spmd(nc, in_maps, core_ids=list(range(NCORES)))
    LAST_RESULT = res
    out = np.zeros((B, S, DM), f32)
    for c in range(NCORES):
        out[c // 4] += np.asarray(res.results[c]["out"], f32)
    out += bo
    return out

